# revision 1
# baseline (speedup 1.0000x reference)
"""Trainium2 Bass kernel for nn_Attention_49598282334528.

Dense transformer attention block: fused QKV projection + RoPE + causal
GQA flash-style attention + output projection, for
  x: [2, 2048, 2048], H=16 q heads, KV=4 kv heads, head_dim=128.

Sharding (8 NeuronCores): data-parallel over batch (2) x tensor-parallel
over kv-head groups (4).  Core c handles batch c//4, kv-group c%4 (4 q
heads + 1 kv head).  Each core computes a full-width partial of the
output projection (row-parallel Wo); the host sums the 4 partials per
batch (the unshard for row-parallel sharding) and stacks batches.

On-device layout choices (all matmuls float32r = full-rate fp32):
  - host ships xT = x[b].T so the d-contraction sits on partitions
  - q/k are projected directly transposed (qT/kT: [head_dim, seq]) with
    RoPE applied in transposed space; the rope pair interleave is
    pre-permuted into the Wq/Wk columns host-side (even dims then odd
    dims), which turns RoPE into half-partition tensor ops.  Scores are
    invariant to a shared permutation of q/k head dims.
  - v is projected transposed then PE-transposed back to natural [s, e]
    so the PV matmul (lhsT=v chunk, rhs=exp(ST)) directly yields
    attention output transposed [e, s], which feeds the output
    projection with no further transposes.
  - scores are built transposed: ST[sk, sq] = kT.T @ qT; softmax
    denominators come from an all-ones matmul accumulated alongside PV.
"""

import sys

if "/opt/trn_rl_repo" not in sys.path:
    sys.path.insert(0, "/opt/trn_rl_repo")

import numpy as np

B, S, D = 2, 2048, 2048
H, KV, HD = 16, 4, 128
G = 4                # kv groups == cores per batch
QPH = H // KV        # q heads per group = 4
EQ = QPH * HD        # per-core q width = 512
NCORES = 8
P = 128
SCALE = 1.0 / float(np.sqrt(HD))

_CACHE = {}


def _build_program():
    import concourse.bass as bass
    import concourse.tile as tile
    from concourse import bacc, mybir

    f32 = mybir.dt.float32
    f32r = mybir.dt.float32r
    EXP = mybir.ActivationFunctionType.Exp
    COPY = mybir.ActivationFunctionType.Copy

    nc = bacc.Bacc("TRN2", target_bir_lowering=False, debug=False)

    xt = nc.dram_tensor("xt", [D, S], f32r, kind="ExternalInput").ap()
    wq = nc.dram_tensor("wq", [D, EQ], f32r, kind="ExternalInput").ap()
    wk = nc.dram_tensor("wk", [D, HD], f32r, kind="ExternalInput").ap()
    wv = nc.dram_tensor("wv", [D, HD], f32r, kind="ExternalInput").ap()
    wo = nc.dram_tensor("wo", [EQ, D], f32r, kind="ExternalInput").ap()
    cosT = nc.dram_tensor("cosT", [HD // 2, S], f32, kind="ExternalInput").ap()
    sinT = nc.dram_tensor("sinT", [HD // 2, S], f32, kind="ExternalInput").ap()
    ones_d = nc.dram_tensor("ones_d", [P, P], f32r, kind="ExternalInput").ap()
    ident_d = nc.dram_tensor("ident_d", [P, P], f32r, kind="ExternalInput").ap()
    maska_d = nc.dram_tensor("maska_d", [P, P], f32r, kind="ExternalInput").ap()
    outp = nc.dram_tensor("outp", [S, D], f32, kind="ExternalOutput").ap()

    xt_r = xt.rearrange("(o p) s -> p o s", p=P)     # [128, 16, 2048]
    wq_r = wq.rearrange("(o p) e -> p o e", p=P)     # [128, 16, 512]
    wk_r = wk.rearrange("(o p) e -> p o e", p=P)     # [128, 16, 128]
    wv_r = wv.rearrange("(o p) e -> p o e", p=P)     # [128, 16, 128]
    wo_r = wo.rearrange("(h p) d -> p h d", p=P)     # [128, 4, 2048]

    ND = D // P          # 16 contraction chunks
    ABLK = 512           # phase-A seq block width (also attention block)
    NA = S // ABLK       # 4
    BBLK = 512           # phase-B sq block width
    NB = S // BBLK       # 4
    NS = S // P          # 16 sk tiles

    with tile.TileContext(nc) as tc:
        import contextlib

        # ---- persistent tensors (span all phases) ----
        with contextlib.ExitStack() as stack:
            const = stack.enter_context(tc.tile_pool(name="const", bufs=1))
            persist = stack.enter_context(tc.tile_pool(name="persist", bufs=1))

            cos_sb = const.tile([HD // 2, S], f32)
            sin_sb = const.tile([HD // 2, S], f32)
            ones_sb = const.tile([P, P], f32r)
            ident_sb = const.tile([P, P], f32r)
            maska_sb = const.tile([P, P], f32r)
            nc.sync.dma_start(out=cos_sb[:], in_=cosT[:])
            nc.sync.dma_start(out=sin_sb[:], in_=sinT[:])
            nc.sync.dma_start(out=ones_sb[:], in_=ones_d[:])
            nc.sync.dma_start(out=ident_sb[:], in_=ident_d[:])
            nc.sync.dma_start(out=maska_sb[:], in_=maska_d[:])


            # per-attention-block tensors: separate tiles let the
            # scheduler overlap projection, attention, and output phases
            # (deps are tracked per tile)
            qT_blks = [persist.tile([P, QPH, ABLK], f32r, name=f"qTb{b}")
                       for b in range(NA)]
            kT_blks = [persist.tile([P, ABLK], f32r, name=f"kTb{b}")
                       for b in range(NA)]
            v_blks = [persist.tile([P, ABLK // P, HD], f32r, name=f"vb{b}")
                      for b in range(NA)]

            # ================= phase A: QKV projection + rope =============
            with contextlib.ExitStack() as pa:
                wpool = pa.enter_context(tc.tile_pool(name="wproj", bufs=1))
                xtp = pa.enter_context(tc.tile_pool(name="xtp", bufs=2))
                aps = pa.enter_context(
                    tc.tile_pool(name="apsum", bufs=1, space="PSUM")
                )
                tmpp = pa.enter_context(tc.tile_pool(name="ropetmp", bufs=3))

                wq_sb = wpool.tile([P, ND, EQ], f32r)
                wk_sb = wpool.tile([P, ND, HD], f32r)
                wv_sb = wpool.tile([P, ND, HD], f32r)
                nc.sync.dma_start(out=wq_sb[:], in_=wq_r[:])
                nc.sync.dma_start(out=wk_sb[:], in_=wk_r[:])
                nc.sync.dma_start(out=wv_sb[:], in_=wv_r[:])

                for blk in range(NA):
                    s0 = blk * ABLK
                    xt_t = xtp.tile([P, ND, ABLK], f32r, tag="xt")
                    nc.sync.dma_start(
                        out=xt_t[:], in_=xt_r[:, :, s0 : s0 + ABLK]
                    )
                    qps = [
                        aps.tile([P, ABLK], f32, tag=f"ps{j}", name=f"ps{j}") for j in range(6)
                    ]
                    for di in range(ND):
                        rhs = xt_t[:, di, :]
                        st = di == 0
                        sp = di == ND - 1
                        for h in range(QPH):
                            nc.tensor.matmul(
                                qps[h][:],
                                wq_sb[:, di, h * HD : (h + 1) * HD],
                                rhs,
                                start=st,
                                stop=sp,
                            )
                        nc.tensor.matmul(
                            qps[4][:], wk_sb[:, di, :], rhs,
                            start=st, stop=sp,
                        )
                        nc.tensor.matmul(
                            qps[5][:], wv_sb[:, di, :], rhs,
                            start=st, stop=sp,
                        )

                    # rope for q heads and k; writes into qT_sb/kT_sb
                    HH = HD // 2
                    ct = cos_sb[:, s0 : s0 + ABLK]
                    st_ = sin_sb[:, s0 : s0 + ABLK]
                    rope_jobs = [(qps[h], qT_blks[blk][:, h, :]) for h in range(QPH)]
                    rope_jobs.append((qps[4], kT_blks[blk][:]))
                    for src, dst in rope_jobs:
                        top = src[0:HH, :]
                        bot = src[HH:P, :]
                        t1 = tmpp.tile([HH, ABLK], f32, tag="t1", name="t1")
                        t2 = tmpp.tile([HH, ABLK], f32, tag="t2", name="t2")
                        nc.vector.tensor_mul(t1[:], top, ct)
                        nc.vector.tensor_mul(t2[:], bot, st_)
                        nc.vector.tensor_sub(dst[0:HH, :], t1[:], t2[:])
                        t3 = tmpp.tile([HH, ABLK], f32, tag="t1", name="t1")
                        t4 = tmpp.tile([HH, ABLK], f32, tag="t2", name="t2")
                        nc.vector.tensor_mul(t3[:], top, st_)
                        nc.vector.tensor_mul(t4[:], bot, ct)
                        nc.vector.tensor_add(dst[HH:P, :], t3[:], t4[:])

                    # v: evict transposed psum to sbuf, PE-transpose back
                    vt_t = tmpp.tile([P, ABLK], f32r, tag="vt", name="vt")
                    nc.scalar.activation(vt_t[:], qps[5][:], COPY)
                    for j in range(ABLK // P):
                        vps = aps.tile([P, P], f32r, tag="vtp", name="vtp")
                        nc.tensor.transpose(
                            vps[:], vt_t[:, j * P : (j + 1) * P], ident_sb[:]
                        )
                        nc.scalar.activation(v_blks[blk][:, j, :], vps[:], COPY)

            # ============ phase B: causal attention (transposed) ==========
            with contextlib.ExitStack() as pb:
                wop = pb.enter_context(tc.tile_pool(name="wop", bufs=1))
                wo_sb = wop.tile([P, QPH, D], f32r)
                nc.sync.dma_start(out=wo_sb[:], in_=wo_r[:])

                pbi = pb.enter_context(contextlib.ExitStack())
                stps = pbi.enter_context(
                    tc.tile_pool(name="stpsum", bufs=3, space="PSUM")
                )
                accps = pbi.enter_context(
                    tc.tile_pool(name="accpsum", bufs=2, space="PSUM")
                )
                stsb = pbi.enter_context(tc.tile_pool(name="stsb", bufs=4))
                nrm = pbi.enter_context(tc.tile_pool(name="nrm", bufs=2))

                oT_blks = [wop.tile([P, QPH, ABLK], f32r, name=f"oTb{b}")
                           for b in range(NA)]
                for blk in range(NB):
                    s0 = blk * BBLK
                    n_sk = (s0 + BBLK) // P
                    for h in range(QPH):
                        qT_blk = qT_blks[blk][:, h, :]
                        oT_ps = accps.tile([P, BBLK], f32, tag="oT",
                                           name="oTps")
                        sm_ps = accps.tile([P, BBLK], f32, tag="sums",
                                           name="smps")
                        for ki in range(n_sk):
                            lead = max(ki * P - s0, 0)
                            diag = 0 <= ki * P - s0 < BBLK
                            st_ps = stps.tile([P, BBLK], f32, tag="st",
                                              name="stps_t")
                            nc.tensor.matmul(
                                st_ps[:],
                                kT_blks[ki // (ABLK // P)][
                                    :, (ki % (ABLK // P)) * P
                                    : (ki % (ABLK // P) + 1) * P],
                                qT_blk,
                                start=True,
                                stop=not diag,
                            )
                            if diag:
                                # add -1e30 above the diagonal via the PE so
                                # the exp sees only PE-written data
                                nc.tensor.matmul(
                                    st_ps[:, lead : lead + P],
                                    ident_sb[:], maska_sb[:],
                                    start=False, stop=True,
                                )
                            st_t = stsb.tile([P, BBLK], f32r, tag="stsb",
                                             name="stsb_t")
                            nc.scalar.activation(
                                st_t[:, lead:], st_ps[:, lead:], EXP,
                                scale=SCALE,
                            )
                            first = ki == 0
                            last = ki == n_sk - 1
                            nc.tensor.matmul(
                                oT_ps[:, lead:],
                                v_blks[ki // (ABLK // P)][
                                    :, ki % (ABLK // P), :],
                                st_t[:, lead:],
                                start=first, stop=last,
                            )
                            nc.tensor.matmul(
                                sm_ps[:, lead:],
                                ones_sb[:],
                                st_t[:, lead:],
                                start=first, stop=last,
                            )
                        rc = nrm.tile([P, BBLK], f32, tag="rc", name="rc")
                        nc.vector.reciprocal(rc[:], sm_ps[:])
                        nc.vector.tensor_mul(
                            oT_blks[blk][:, h, :], oT_ps[:], rc[:]
                        )

                # ========== phase C: output projection (partial) ==========
                pbi.close()
                with tc.tile_pool(name="opsum", bufs=4, space="PSUM") as ops, \
                        tc.tile_pool(name="ostage", bufs=4) as osg:
                    for t in range(S // P):
                        for cblk in range(D // 512):
                            op_ps = ops.tile([P, 512], f32, tag="op", name="opps")
                            bt, off = divmod(t, ABLK // P)
                            for h in range(QPH):
                                nc.tensor.matmul(
                                    op_ps[:],
                                    oT_blks[bt][:, h, off * P : (off + 1) * P],
                                    wo_sb[:, h, cblk * 512 : (cblk + 1) * 512],
                                    start=(h == 0),
                                    stop=(h == QPH - 1),
                                )
                            op_sb = osg.tile([P, 512], f32, tag="opsb",
                                             name="opsb")
                            nc.scalar.activation(op_sb[:], op_ps[:], COPY)
                            nc.sync.dma_start(
                                out=outp[t * P : (t + 1) * P,
                                         cblk * 512 : (cblk + 1) * 512],
                                in_=op_sb[:],
                            )

    _strip_pe_self_waits(nc)
    nc.finalize()
    return nc


def _strip_pe_self_waits(nc):
    """Remove PE-on-PE semaphore waits from PE matmuls.

    Tile's semaphore assigner emits conservative same-proc waits for
    PSUM-bank WAW reuse.  They are always satisfied by program order (PE
    matmuls complete strictly in order, and ldweights pull-ahead only
    reads SBUF, which PE never writes), but the self-loading fp32r
    matmul form (S3_LW) has a single sync-wait slot, so a matmul
    carrying {PE self-wait + real cross-engine wait} fails walrus
    codegen.  Stripping the self-wait is hardware-safe and frees the
    slot for the real dependency.
    """
    import concourse.mybir as mybir

    stripped = 0
    for bb in nc.m.functions[0].blocks:
        for inst in bb.instructions:
            si = getattr(inst, "sync_info", None)
            if si is None or not getattr(si, "on_wait", None):
                continue
            if isinstance(inst, mybir.InstMatmult):
                keep = [
                    w for w in si.on_wait
                    if not (w.sync_type == "semaphore"
                            and w.ant_name.startswith("PE"))
                ]
                stripped += len(si.on_wait) - len(keep)
                si.on_wait = keep
    return stripped


def _prep_inputs(x, freqs_cos, freqs_sin, Wq, Wk, Wv, Wo):
    """Build the 8 per-core input maps (pure layout work, no arithmetic)."""
    perm = np.concatenate([np.arange(0, HD, 2), np.arange(1, HD, 2)])

    cosT = np.ascontiguousarray(freqs_cos.T.astype(np.float32))  # [64, S]
    sinT = np.ascontiguousarray(freqs_sin.T.astype(np.float32))
    ones = np.ones((P, P), np.float32)
    ident = np.eye(P, dtype=np.float32)
    # [sk, sq]: 0 where sk <= sq (keep), -1e30 above-diagonal (mask)
    maska = np.where(np.triu(np.ones((P, P), bool)), 0.0, -1e30).astype(
        np.float32
    )

    xTs = [np.ascontiguousarray(x[b].T.astype(np.float32)) for b in range(B)]

    wqs, wks, wvs, wos = [], [], [], []
    for g in range(G):
        wq_g = Wq[:, g * EQ : (g + 1) * EQ].reshape(D, QPH, HD)[:, :, perm]
        wqs.append(np.ascontiguousarray(wq_g.reshape(D, EQ), dtype=np.float32))
        wk_g = Wk[:, g * HD : (g + 1) * HD][:, perm]
        wks.append(np.ascontiguousarray(wk_g, dtype=np.float32))
        wvs.append(np.ascontiguousarray(Wv[:, g * HD : (g + 1) * HD],
                                        dtype=np.float32))
        wos.append(np.ascontiguousarray(Wo[g * EQ : (g + 1) * EQ, :],
                                        dtype=np.float32))

    in_maps = []
    for c in range(NCORES):
        b, g = divmod(c, G)
        in_maps.append(
            dict(xt=xTs[b], wq=wqs[g], wk=wks[g], wv=wvs[g], wo=wos[g],
                 cosT=cosT, sinT=sinT, ones_d=ones, ident_d=ident,
                 maska_d=maska)
        )
    return in_maps


LAST_RESULTS = None


def kernel(**inputs) -> np.ndarray:
    global LAST_RESULTS
    x = np.asarray(inputs["x"], np.float32)
    in_maps = _prep_inputs(
        x,
        np.asarray(inputs["freqs_cos"], np.float32),
        np.asarray(inputs["freqs_sin"], np.float32),
        np.asarray(inputs["Wq"], np.float32),
        np.asarray(inputs["Wk"], np.float32),
        np.asarray(inputs["Wv"], np.float32),
        np.asarray(inputs["Wo"], np.float32),
    )

    if "nc" not in _CACHE:
        _CACHE["nc"] = _build_program()
    nc = _CACHE["nc"]

    from concourse import bass_utils

    res = bass_utils.run_bass_kernel_spmd(nc, in_maps, list(range(NCORES)))
    LAST_RESULTS = res

    out = np.empty((B, S, D), np.float32)
    for b in range(B):
        acc = res.results[4 * b]["outp"].astype(np.float32)
        for g in range(1, G):
            acc = acc + res.results[4 * b + g]["outp"]
        out[b] = acc
    return out

